# revision 48
# baseline (speedup 1.0000x reference)
"""Trainium2 Bass kernel for nn_DecoderLayer (dense transformer decoder layer).

Strategy (8 NeuronCores, full inputs in / full output out):
  - core c handles batch b = c//4 and token-quarter r = c%4 (rows [r*W, (r+1)*W)).
  - All phases are token-sharded: each core computes Q/K/V, attention,
    projections, LayerNorms and FFN for its W tokens only.
  - K/V are computed non-redundantly (each core projects only its own W tokens)
    and exchanged via ONE bf16 AllGather per attention block within the 4-core
    batch group. Gathered K^T / V live in SBUF for the whole attention phase
    (no DRAM spill/reload). This also removes the x1 AllGather entirely:
    block-2 K/V are projected from the local x1 slice and gathered the same way.
  - Attention per head: S^T[k, q] = K_h^T-slices.T @ Q_h^T (scores transposed,
    fp32r), exp on ACT (1/8 scale fused) with bf16 output, causal/arbitrary
    mask as multiplicative bf16 [128, W] tiles (per-core mask DATA), softmax
    denominators free via a ones-column on V, AV matmuls in bf16.
  - Normalization is deferred and batched: denominators for all heads are
    collected into one [H, W] tile, ONE vector reciprocal, then per head-pair
    a selector-matmul broadcasts 1/den to [128, W] on the PE and a single DVE
    multiply normalizes the attention output in place.
  - LayerNorm in transposed layout: cross-partition sums via ones-matmul on
    the PE, stats broadcast back via ones-matmul.
"""

import sys

if "/opt/trn_rl_repo" not in sys.path:
    sys.path.insert(0, "/opt/trn_rl_repo")

import numpy as np

P = 128
HD = 64
HD1 = HD + 1
EPS = 1e-5


class Cfg:
    def __init__(self, B=2, S=2048, D=1024, H=16, DFF=4096):
        self.B, self.S, self.D, self.H, self.DFF = B, S, D, H, DFF
        self.W = S // 4             # local tokens per core
        self.DT = D // P            # feature-dim tiles
        self.NTS = S // P           # global sequence tiles (keys)
        self.NTL = self.W // P      # local token tiles
        self.FT = DFF // P          # ffn hidden tiles
        self.HP = min(2, H)         # heads per partition-tile
        self.PR = H // self.HP      # head pairs (partition-tile groups)
        self.KTG = min(2, self.NTS)  # k-tiles per exp group
        self.NG = self.NTS // self.KTG
        self.VCW = min(512, D)      # v-proj dout chunk width
        self.VCN = D // self.VCW
        assert D == H * HD
        assert self.W % P == 0 and D % P == 0 and DFF % P == 0 and S % P == 0


class Flags:
    def __init__(self):
        self.qkb1 = self.vb1 = self.ob1 = False
        self.qkb2 = self.vb2 = self.ob2 = False
        self.fb1 = self.fb2 = False
        self.g1 = self.b1 = self.g2 = self.b2 = self.g3 = self.b3 = False
        self.m1 = True      # trg mask multiplicative tiles
        self.causal = False  # trg mask is tile-constant outside own quarter
        self.pa2 = False     # block-2 phase A (own-quarter via group bias)
        self.kb2 = False    # enc mask additive per-k bias


def _build(nc, tc, cfg, fl):
    import concourse.bass as bass
    import concourse.mybir as mybir
    import concourse.tile as tile  # noqa: F401
    from contextlib import ExitStack

    AF = mybir.ActivationFunctionType
    f32 = mybir.dt.float32
    f32r = mybir.dt.float32r
    bf16 = mybir.dt.bfloat16

    def r32(ap):
        return ap.bitcast(f32r)

    B, S, D, H, DFF = cfg.B, cfg.S, cfg.D, cfg.H, cfg.DFF
    W, DT, NTS, NTL, FT = cfg.W, cfg.DT, cfg.NTS, cfg.NTL, cfg.FT
    HP, PR, KTG, NG = cfg.HP, cfg.PR, cfg.KTG, cfg.NG
    VCW, VCN = cfg.VCW, cfg.VCN
    HPC = VCW // HD  # heads per v-chunk

    # ---------------- DRAM parameters ----------------
    def din(name, shape, dt=f32):
        return nc.dram_tensor(name, shape, dt, kind="ExternalInput").ap()

    xTl = din("xTl", [D, W])
    qkvwT1 = din("qkvwT1", [D, 3 * D], bf16)
    qkvwT2 = din("qkvwT2", [D, 3 * D], bf16)
    owT1 = din("owT1", [D, D], bf16)
    owT2 = din("owT2", [D, D], bf16)
    w1T = din("w1T", [D, DFF], bf16)
    w2T = din("w2T", [DFF, D], bf16)
    selm = din("selm", [H, PR * P])
    m1 = din("m1", [NG, P, KTG * W], bf16) if (fl.m1 and not fl.causal) else None
    mloc = din("mloc", [P, NTL * W], bf16) if fl.m1 else None
    cb1 = din("cb1", [P, NG]) if fl.causal else None
    cb2 = din("cb2", [P, NG]) if fl.pa2 else None
    kb2 = din("kb2", [NTS, P, 1]) if fl.kb2 else None
    qkvb1 = din("qkvb1", [3 * D]) if fl.qkb1 else None
    qkvb2 = din("qkvb2", [3 * D]) if fl.qkb2 else None
    vb1 = din("vb1", [P, D]) if fl.vb1 else None
    vb2 = din("vb2", [P, D]) if fl.vb2 else None
    ob1 = din("ob1", [D]) if fl.ob1 else None
    ob2 = din("ob2", [D]) if fl.ob2 else None
    fb1d = din("fb1", [DFF]) if fl.fb1 else None
    fb2d = din("fb2", [D]) if fl.fb2 else None
    lnp = {}
    for nm, use in [("g1", fl.g1), ("b1", fl.b1), ("g2", fl.g2),
                    ("b2", fl.b2), ("g3", fl.g3), ("b3", fl.b3)]:
        lnp[nm] = din(nm, [D]) if use else None
    out = nc.dram_tensor("out", [D, W], f32, kind="ExternalOutput").ap()

    # collective staging (bf16): [K^T (D*W) | V natural (W*D)] per block
    KVSZ = 2 * D * W
    kv_in1 = nc.dram_tensor("kv_in1", [KVSZ], bf16, kind="Internal").ap()
    kv_out1 = nc.dram_tensor("kv_out1", [4 * KVSZ], bf16, kind="Internal").ap()
    kv_in2 = nc.dram_tensor("kv_in2", [KVSZ], bf16, kind="Internal").ap()
    kv_out2 = nc.dram_tensor("kv_out2", [4 * KVSZ], bf16, kind="Internal").ap()
    groups = [[0, 1, 2, 3], [4, 5, 6, 7]]

    es = ExitStack()
    with es:
        const = es.enter_context(tc.tile_pool(name="const", bufs=1))
        ones_p1 = const.tile([P, 1], f32)
        nc.vector.memset(ones_p1[:, :], 1.0)
        ones_1p = const.tile([1, P], f32)
        nc.vector.memset(ones_1p[0:1, :], 1.0)
        eps_t = const.tile([1, 1], f32)
        nc.vector.memset(eps_t[0:1, :], EPS)
        selsb = const.tile([H, PR * P], f32)
        nc.sync.dma_start(out=r32(selsb[0:H, :]), in_=r32(selm[:, :]))

        def ldvec(dram_vec, n_tiles, name):
            """[D]-style vector -> [P, n_tiles] sbuf tile (per-partition slices)."""
            t = const.tile([P, n_tiles], f32, tag=name)
            nc.sync.dma_start(
                out=t[:, :],
                in_=dram_vec.rearrange("(t p) -> p t", p=P),
            )
            return t

        qkb1sb = ldvec(qkvb1[0 : 2 * D], 2 * DT, "qkb1") if fl.qkb1 else None
        qkb2sb = ldvec(qkvb2[0 : 2 * D], 2 * DT, "qkb2") if fl.qkb2 else None
        ob1sb = ldvec(ob1, DT, "ob1") if fl.ob1 else None
        ob2sb = ldvec(ob2, DT, "ob2") if fl.ob2 else None
        fb1sb = ldvec(fb1d, FT, "fb1") if fl.fb1 else None
        fb2sb = ldvec(fb2d, DT, "fb2") if fl.fb2 else None
        lns = {k: (ldvec(v, DT, "ln" + k) if v is not None else None)
               for k, v in lnp.items()}
        vb1sb = None
        if fl.vb1:
            vb1sb = const.tile([P, D], f32, tag="vb1")
            nc.sync.dma_start(out=vb1sb[:, :], in_=vb1[:, :])
        vb2sb = None
        if fl.vb2:
            vb2sb = const.tile([P, D], f32, tag="vb2")
            nc.sync.dma_start(out=vb2sb[:, :], in_=vb2[:, :])
        cb1sb = None
        if fl.causal:
            cb1sb = const.tile([P, NG], f32, tag="cb1")
            nc.sync.dma_start(out=cb1sb[:, :], in_=cb1[:, :])
        cb2sb = None
        if fl.pa2:
            cb2sb = const.tile([P, NG], f32, tag="cb2")
            nc.sync.dma_start(out=cb2sb[:, :], in_=cb2[:, :])
        kb2sb = None
        if fl.kb2:
            kb2sb = const.tile([P, NTS], f32, tag="kb2")
            nc.sync.dma_start(out=kb2sb[:, :], in_=kb2.rearrange("n p o -> p (n o)"))

        # =========== QKV projection phase (local tokens, all heads) ==========
        def qkv_phase(x_sb, wT, kv_in, qT, klo, vlo, qkb, vbsb):
            """x_sb: [P, DT, W] local tokens (fp32). Computes (order K, V, Q
            so the AllGather can launch as early as possible):
            K^T [D, W] -> klo sbuf [P, DT, W] bf16 + kv_in[0:D*W],
            V natural [W, D] -> vlo sbuf [P, NTL, H, HD1] bf16 + kv_in[D*W:],
            Q^T [D, W] -> qT sbuf (bf16)."""
            DKG = min(2, DT)  # output-tile group per weight-chunk load
            with tc.tile_pool(name="qkv_w", bufs=3) as wp, \
                 tc.tile_pool(name="qkv_st", bufs=3) as stp, \
                 tc.tile_pool(name="qkv_ps", bufs=3, space="PSUM") as psp, \
                 tc.tile_pool(name="qkv_psq", bufs=3, space="PSUM") as psq:
                # ---- K^T [D, W] -> klo + kv_in ----
                for dkg in range(DT // DKG):
                    wsl = wp.tile([P, DT, DKG * P], bf16, tag="wkq")
                    nc.sync.dma_start(
                        out=wsl[:, :, :],
                        in_=wT[:, D + dkg * DKG * P : D + (dkg + 1) * DKG * P]
                        .rearrange("(t p) v -> p t v", p=P),
                    )
                    for j in range(DKG):
                        dk = dkg * DKG + j
                        ps = psp.tile([P, W], f32, tag="kps")
                        for dt in range(DT):
                            nc.tensor.matmul(
                                ps[:, :],
                                lhsT=wsl[:, dt, j * P : (j + 1) * P],
                                rhs=x_sb[:, dt, :],
                                start=(dt == 0),
                                stop=(dt == DT - 1),
                            )
                        if qkb is not None:
                            nc.scalar.activation(
                                out=klo[:, dk, :], in_=ps[:, :],
                                func=AF.Identity,
                                bias=qkb[:, DT + dk : DT + dk + 1], scale=1.0,
                            )
                        else:
                            nc.vector.tensor_copy(klo[:, dk, :], ps[:, :])
                        nc.sync.dma_start(
                            out=kv_in[dk * P * W : (dk + 1) * P * W]
                            .rearrange("(p w) -> p w", p=P),
                            in_=klo[:, dk, :],
                        )
                # ---- V natural [W, D] -> vlo + kv_in ----
                for vc in range(VCN):
                    wsl = wp.tile([P, DT, VCW], bf16, tag="wv")
                    nc.sync.dma_start(
                        out=wsl[:, :, :],
                        in_=wT[:, 2 * D + vc * VCW : 2 * D + (vc + 1) * VCW]
                        .rearrange("(t p) v -> p t v", p=P),
                    )
                    for nt in range(NTL):
                        ps = psq.tile([P, VCW], f32, tag="vps")
                        for dt in range(DT):
                            nc.tensor.matmul(
                                ps[:, :],
                                lhsT=x_sb[:, dt, nt * P : (nt + 1) * P],
                                rhs=wsl[:, dt, :],
                                start=(dt == 0),
                                stop=(dt == DT - 1),
                            )
                        vdst = vlo[:, nt, vc * HPC : (vc + 1) * HPC, 0:HD]
                        if vbsb is not None:
                            tmpv = stp.tile([P, VCW], f32, tag="vtmp")
                            nc.vector.tensor_add(
                                tmpv[:, :], ps[:, :],
                                vbsb[:, vc * VCW : (vc + 1) * VCW],
                            )
                            nc.vector.tensor_copy(
                                vdst, tmpv[:, :].rearrange("p (h d) -> p h d",
                                                           d=HD))
                        else:
                            nc.vector.tensor_copy(
                                vdst,
                                ps[:, :].rearrange("p (h d) -> p h d", d=HD))
                for nt in range(NTL):
                    nc.vector.memset(vlo[:, nt, :, HD:HD1], 1.0)
                    nc.sync.dma_start(
                        out=kv_in[D * W + nt * P * D : D * W + (nt + 1) * P * D]
                        .rearrange("(p h d) -> p h d", h=H, d=HD),
                        in_=vlo[:, nt, :, 0:HD],
                    )
                # ---- Q^T [D, W] local (after the gather is in flight) ----
                for dkg in range(DT // DKG):
                    wsl = wp.tile([P, DT, DKG * P], bf16, tag="wkq")
                    nc.sync.dma_start(
                        out=wsl[:, :, :],
                        in_=wT[:, dkg * DKG * P : (dkg + 1) * DKG * P]
                        .rearrange("(t p) v -> p t v", p=P),
                    )
                    for j in range(DKG):
                        dk = dkg * DKG + j
                        ps = psp.tile([P, W], f32, tag="kps")
                        for dt in range(DT):
                            nc.tensor.matmul(
                                ps[:, :],
                                lhsT=wsl[:, dt, j * P : (j + 1) * P],
                                rhs=x_sb[:, dt, :],
                                start=(dt == 0),
                                stop=(dt == DT - 1),
                            )
                        if qkb is not None:
                            nc.scalar.activation(
                                out=qT[:, dk, :], in_=ps[:, :], func=AF.Identity,
                                bias=qkb[:, dk : dk + 1], scale=1.0,
                            )
                        else:
                            nc.vector.tensor_copy(qT[:, dk, :], ps[:, :])

        # =========== gathered K/V -> SBUF ===========
        def load_kv(kv_out, kvp):
            """kv_out: [4*KVSZ] bf16 gathered. Returns (ksb bf16 [P, PR, S],
            vsb bf16 [P, NTS, H, HD1])."""
            kvo = kv_out.rearrange("(g r) -> g r", g=4)
            ksb = kvp.tile([P, PR, 4, W], bf16, tag="ksb")
            for g in range(4):
                nc.sync.dma_start(
                    out=ksb[:, :, g, :],
                    in_=kvo[g, 0 : D * W].rearrange(
                        "(pr p w) -> p pr w", p=P, w=W),
                )
            vsb = kvp.tile([P, NTS, H, HD1], bf16, tag="vsb")
            for g in range(4):
                for ln in range(NTL):
                    nc.sync.dma_start(
                        out=vsb[:, g * NTL + ln, :, 0:HD],
                        in_=kvo[g, D * W + ln * P * D : D * W + (ln + 1) * P * D]
                        .rearrange("(p h hd) -> p h hd", hd=HD, h=H),
                    )
            nc.vector.memset(vsb[:, :, :, HD:HD1], 1.0)
            return ksb, vsb

        # =========== attention phase ===========
        def attn_inner(heads, ktiles, kslc, vslc, mslc, use_kb2, gbias, aoT,
                       den_dst, accumulate, exp_, dtp, psp, pop):
            """Score/exp/mask/AV chain for a PAIR of heads, groups interleaved
            across the heads so the PE sees long back-to-back matmul bursts.
            accumulate=False: aoT <- po; True: aoT += po."""
            ktg = min(KTG, len(ktiles))
            ngl = len(ktiles) // ktg
            pos = [pop.tile([HD1, W], f32, tag="po%d" % i, name="po%d" % i)
                   for i in range(len(heads))]
            for g in range(ngl):
                pss = [psp.tile([P, KTG * W], f32, tag="sc", name="sc")
                       for _ in heads]
                for o in range(ktg):
                    kt = ktiles[g * ktg + o]
                    for hi, h in enumerate(heads):
                        hh = (h % HP) * HD
                        pr = h // HP
                        nc.tensor.matmul(
                            pss[hi][:, o * W : (o + 1) * W],
                            lhsT=kslc(kt, hh, pr),
                            rhs=qT[hh : hh + HD, pr, :],
                            start=True,
                            stop=True,
                        )
                exs = []
                for hi, h in enumerate(heads):
                    ps = pss[hi]
                    ex = exp_.tile([P, KTG * W], bf16, tag="ex", name="ex")
                    if use_kb2:
                        for o in range(ktg):
                            kt = ktiles[g * ktg + o]
                            nc.scalar.activation(
                                out=ex[:, o * W : (o + 1) * W],
                                in_=ps[:, o * W : (o + 1) * W], func=AF.Exp,
                                bias=kb2sb[:, kt : kt + 1],
                                scale=1.0 / np.sqrt(HD),
                            )
                    elif gbias is not None:
                        nc.scalar.activation(
                            out=ex[:, 0 : ktg * W], in_=ps[:, 0 : ktg * W],
                            func=AF.Exp, bias=gbias(g),
                            scale=1.0 / np.sqrt(HD),
                        )
                    else:
                        nc.scalar.activation(
                            out=ex[:, 0 : ktg * W], in_=ps[:, 0 : ktg * W],
                            func=AF.Exp, scale=1.0 / np.sqrt(HD),
                        )
                    if mslc is not None:
                        nc.vector.tensor_mul(ex[:, 0 : ktg * W],
                                             ex[:, 0 : ktg * W], mslc(g, ktg))
                    exs.append(ex)
                for hi, h in enumerate(heads):
                    for o in range(ktg):
                        kt = ktiles[g * ktg + o]
                        nc.tensor.matmul(
                            pos[hi][0:HD1, :],
                            lhsT=vslc(kt, h),
                            rhs=exs[hi][:, o * W : (o + 1) * W],
                            start=(g == 0 and o == 0),
                            stop=(g == ngl - 1 and o == ktg - 1),
                        )
            for hi, h in enumerate(heads):
                hh = (h % HP) * HD
                pr = h // HP
                po = pos[hi]
                if accumulate:
                    nc.vector.tensor_add(aoT[hh : hh + HD, pr, :],
                                         aoT[hh : hh + HD, pr, :], po[0:HD, :])
                else:
                    nc.vector.tensor_copy(aoT[hh : hh + HD, pr, :],
                                          po[0:HD, :])
                dtmp = dtp.tile([1, W], f32, tag="dtmp", name="dtmp")
                nc.vector.tensor_copy(dtmp[0:1, :], po[HD:HD1, :])
                nc.sync.dma_start(out=den_dst[h : h + 1, :], in_=dtmp[0:1, :])

        def attn_local(klo, vlo, mlsb, aoT, den_dst):
            """Phase A: attention against the core's own NTL k-tiles (runs
            while the AllGather is in flight). klo [P, DT, W] bf16,
            vlo [P, NTL, H, HD1] bf16, mlsb [P, NTL*W] bf16 or None."""
            with tc.tile_pool(name="atl_ex", bufs=4) as exp_, \
                 tc.tile_pool(name="atl_dt", bufs=3) as dtp, \
                 tc.tile_pool(name="atl_ps", bufs=3, space="PSUM") as psp, \
                 tc.tile_pool(name="atl_po", bufs=1, space="PSUM") as pop:
                kslc = lambda lt, hh, pr: klo[hh : hh + HD, pr,
                                              lt * P : (lt + 1) * P]
                vslc = lambda lt, h: vlo[:, lt, h, :]
                mslc = None
                if mlsb is not None:
                    mslc = lambda g, ktg: mlsb[:, g * ktg * W : (g + 1) * ktg * W]
                for h0 in range(0, H, 2):
                    attn_inner([h0, h0 + 1] if h0 + 1 < H else [h0],
                               list(range(NTL)), kslc, vslc, mslc, False,
                               None, aoT, den_dst, False, exp_, dtp, psp, pop)

        def attn_phase(ksb, vsb, m1sb, use_kb2, gbias, aoT, den_dst,
                       accumulate):
            """Phase B: attention against all gathered k-tiles.
            m1sb: [P, NG, KTG*W] bf16 or None."""
            with tc.tile_pool(name="at_ex", bufs=4) as exp_, \
                 tc.tile_pool(name="at_dt", bufs=3) as dtp, \
                 tc.tile_pool(name="at_ps", bufs=3, space="PSUM") as psp, \
                 tc.tile_pool(name="at_po", bufs=1, space="PSUM") as pop:
                kslc = lambda kt, hh, pr: ksb[hh : hh + HD, pr, kt // NTL,
                                              (kt % NTL) * P : (kt % NTL + 1) * P]
                vslc = lambda kt, h: vsb[:, kt, h, :]
                mslc = None
                if m1sb is not None:
                    mslc = lambda g, ktg: m1sb[:, g, :]
                for h0 in range(0, H, 2):
                    attn_inner([h0, h0 + 1] if h0 + 1 < H else [h0],
                               list(range(NTS)), kslc, vslc, mslc, use_kb2,
                               gbias, aoT, den_dst, accumulate, exp_, dtp,
                               psp, pop)

        def attn_norm(aoT, den_a, den_b, dnp_const):
            # batched normalize: dinv = 1/den, broadcast per head-pair via PE
            dinv = dnp_const.tile([H, W], f32, tag="dinv")
            if den_a is not None:
                dsum = dnp_const.tile([H, W], f32, tag="dsum")
                nc.vector.tensor_add(dsum[0:H, :], den_a[0:H, :], den_b[0:H, :])
                den_fin = dsum
            else:
                den_fin = den_b
            with nc.allow_low_precision(reason="fp32r rounding of 1/den"):
                nc.vector.reciprocal(r32(dinv[0:H, :]), den_fin[0:H, :])
            with tc.tile_pool(name="at_bc", bufs=2, space="PSUM") as bcp:
                for pr in range(PR):
                    bc = bcp.tile([P, W], f32, tag="bc")
                    nc.tensor.matmul(
                        bc[:, :],
                        lhsT=r32(selsb[0:H, pr * P : (pr + 1) * P]),
                        rhs=r32(dinv[0:H, :]),
                        start=True, stop=True,
                    )
                    nc.vector.tensor_mul(aoT[:, pr, :], aoT[:, pr, :],
                                         bc[:, :])

        # =========== layernorm (transposed layout) ===========
        def ln_stats_step(pre, d, acc, sqa, lp):
            """Incremental LN stats: call right after pre[:, d, :] is ready."""
            sqt = lp.tile([P, W], f32, tag="lnsqt")
            nc.vector.tensor_mul(sqt[:, :], pre[:, d, :], pre[:, d, :])
            if d == 0:
                nc.vector.tensor_copy(acc[:, :], pre[:, 0, :])
                nc.vector.tensor_copy(sqa[:, :], sqt[:, :])
            else:
                nc.vector.tensor_add(acc[:, :], acc[:, :], pre[:, d, :])
                nc.vector.tensor_add(sqa[:, :], sqa[:, :], sqt[:, :])

        def ln_t(pre, out_t, g_sb, b_sb, lpp, lp, round_out=True,
                 acc=None, sqa=None):
            if out_t.tensor.dtype != f32:
                round_out = False
            ro = r32 if round_out else (lambda ap: ap)
            if acc is None:
                acc = lp.tile([P, W], f32, tag="lnacc")
                sqa = lp.tile([P, W], f32, tag="lnsqa")
                for d in range(DT):
                    ln_stats_step(pre, d, acc, sqa, lp)
            sums = lpp.tile([1, W], f32, tag="lnsums")
            nc.tensor.matmul(sums[0:1, :], lhsT=ones_p1[:, :],
                             rhs=acc[:, :], start=True, stop=True)
            sqs = lpp.tile([1, W], f32, tag="lnsqs")
            nc.tensor.matmul(sqs[0:1, :], lhsT=ones_p1[:, :],
                             rhs=sqa[:, :], start=True, stop=True)
            mu = lp.tile([1, W], f32, tag="lnmu")
            nc.vector.tensor_scalar_mul(mu[0:1, :], sums[0:1, :], 1.0 / D)
            ex2 = lp.tile([1, W], f32, tag="lnex2")
            nc.vector.tensor_scalar_mul(ex2[0:1, :], sqs[0:1, :], 1.0 / D)
            mu2 = lp.tile([1, W], f32, tag="lnmu2")
            nc.scalar.square(mu2[0:1, :], mu[0:1, :])
            var = lp.tile([1, W], f32, tag="lnvar")
            nc.vector.tensor_sub(var[0:1, :], ex2[0:1, :], mu2[0:1, :])
            sd = lp.tile([1, W], f32, tag="lnsd")
            nc.scalar.activation(out=sd[0:1, :], in_=var[0:1, :], func=AF.Sqrt,
                                 bias=eps_t[0:1, :], scale=1.0)
            rstd = lp.tile([1, W], f32, tag="lnrstd")
            nc.vector.reciprocal(rstd[0:1, :], sd[0:1, :])
            mub = lpp.tile([P, W], f32, tag="lnmub")
            nc.tensor.matmul(mub[:, :], lhsT=ones_1p[0:1, :],
                             rhs=mu[0:1, :], start=True, stop=True)
            rstdb = lpp.tile([P, W], f32, tag="lnrstdb")
            nc.tensor.matmul(rstdb[:, :], lhsT=ones_1p[0:1, :],
                             rhs=rstd[0:1, :], start=True, stop=True)
            for d in range(DT):
                t1 = lp.tile([P, W], f32, tag="lnt1")
                nc.vector.tensor_sub(t1[:, :], pre[:, d, :], mub[:, :])
                nc.vector.tensor_mul(ro(out_t[:, d, :]), t1[:, :], rstdb[:, :])
                if g_sb is not None:
                    nc.vector.tensor_scalar_mul(
                        ro(out_t[:, d, :]), out_t[:, d, :], g_sb[:, d : d + 1])
                if b_sb is not None:
                    nc.vector.tensor_scalar_add(
                        ro(out_t[:, d, :]), out_t[:, d, :], b_sb[:, d : d + 1])

        # =========== out-projection + residual + LN ===========
        def proj_preload(owT, wp):
            wall = wp.tile([P, DT, D], bf16, tag="prw")
            nc.sync.dma_start(
                out=wall[:, :, :],
                in_=owT.rearrange("(t p) v -> p t v", p=P),
            )
            return wall

        def proj_resid_ln(wall, obsb, aoT, residT, g_sb, b_sb, out_t):
            with tc.tile_pool(name="pr_t", bufs=2) as lp, \
                 tc.tile_pool(name="pr_pre", bufs=1) as prep, \
                 tc.tile_pool(name="pr_ps", bufs=2, space="PSUM") as psp, \
                 tc.tile_pool(name="pr_lnps", bufs=1, space="PSUM") as lpp:
                pre = prep.tile([P, DT, W], f32, tag="pre")
                pacc = prep.tile([P, W], f32, tag="pracc")
                psqa = prep.tile([P, W], f32, tag="prsqa")
                G4 = min(4, DT)
                for dg in range(DT // G4):
                    for j in range(G4):
                        d = dg * G4 + j
                        ps = psp.tile([P, W], f32, tag="prps")
                        for dt in range(DT):
                            nc.tensor.matmul(
                                ps[:, :],
                                lhsT=wall[:, dt, d * P : (d + 1) * P],
                                rhs=aoT[:, dt, :],
                                start=(dt == 0), stop=(dt == DT - 1),
                            )
                        if obsb is not None:
                            tmp = lp.tile([P, W], f32, tag="prtmp")
                            nc.scalar.activation(out=tmp[:, :], in_=ps[:, :],
                                                 func=AF.Identity,
                                                 bias=obsb[:, d : d + 1], scale=1.0)
                            nc.vector.tensor_add(pre[:, d, :], tmp[:, :],
                                                 residT[:, d, :])
                        else:
                            nc.vector.tensor_add(pre[:, d, :], ps[:, :],
                                                 residT[:, d, :])
                        ln_stats_step(pre, d, pacc, psqa, lp)
                ln_t(pre, out_t, g_sb, b_sb, lpp, lp, acc=pacc, sqa=psqa)

        # ================= pipeline =================
        midp = es.enter_context(tc.tile_pool(name="mid", bufs=1))
        qT = midp.tile([P, DT, W], bf16)     # Q^T local (reused block2)
        aoT = midp.tile([P, PR, W], bf16)    # attention out^T (reused)
        x1T = midp.tile([P, DT, W], f32)     # x1 local
        x2T = midp.tile([P, DT, W], f32)     # x2 local
        dnp = es.enter_context(tc.tile_pool(name="dn", bufs=1))

        den_a = dnp.tile([H, W], f32, tag="dena")
        den_b = dnp.tile([H, W], f32, tag="denb")

        with tc.tile_pool(name="xtl", bufs=1) as xtlp:
            xTlt = xtlp.tile([P, DT, W], f32)
            nc.sync.dma_start(out=r32(xTlt[:, :, :]),
                              in_=r32(xTl.rearrange("(t p) s -> p t s", p=P)))
            xtlb = xtlp.tile([P, DT, W], bf16)
            nc.vector.tensor_copy(
                xtlb[:, :, :].rearrange("p t w -> p (t w)"),
                xTlt[:, :, :].rearrange("p t w -> p (t w)"))

            with tc.tile_pool(name="klv1", bufs=1) as klvp:
                klo = klvp.tile([P, DT, W], bf16, tag="klo")
                vlo = klvp.tile([P, NTL, H, HD1], bf16, tag="vlo")
                qkv_phase(xtlb, qkvwT1, kv_in1, qT, klo, vlo, qkb1sb, vb1sb)
                nc.gpsimd.collective_compute(
                    "AllGather", bass.mybir.AluOpType.bypass,
                    replica_groups=groups,
                    ins=[kv_in1[:]], outs=[kv_out1[:]],
                )
                # phase A: local k-tiles, overlaps the AllGather (needs
                # mask data to zero the own quarter in phase B)
                if fl.m1:
                    with tc.tile_pool(name="mlp", bufs=1) as mlp:
                        mlsb = mlp.tile([P, NTL * W], bf16)
                        nc.sync.dma_start(out=mlsb[:, :], in_=mloc[:, :])
                        attn_local(klo, vlo, mlsb, aoT, den_a)
            # phase B: gathered k-tiles (own quarter zeroed via mask data)
            with tc.tile_pool(name="kv1", bufs=1) as kvp:
                ksb, vsb = load_kv(kv_out1, kvp)
                if fl.causal:
                    attn_phase(ksb, vsb, None, False,
                               lambda g: cb1sb[:, g : g + 1], aoT, den_b, True)
                elif fl.m1:
                    with tc.tile_pool(name="m1p", bufs=1) as m1p:
                        m1sb = m1p.tile([P, NG, KTG * W], bf16)
                        nc.sync.dma_start(
                            out=m1sb[:, :, :],
                            in_=m1.rearrange("g p w -> p g w"))
                        attn_phase(ksb, vsb, m1sb, False, None, aoT, den_b,
                                   True)
                else:
                    attn_phase(ksb, vsb, None, False, None, aoT, den_b, False)
            attn_norm(aoT, den_a if fl.m1 else None, den_b, dnp)

            with tc.tile_pool(name="pr_w1", bufs=1) as wp1:
                ow1w = proj_preload(owT1, wp1)
                proj_resid_ln(ow1w, ob1sb, aoT, xTlt, lns["g1"], lns["b1"],
                              x1T)

        with tc.tile_pool(name="klv2", bufs=1) as klvp:
            klo2 = klvp.tile([P, DT, W], bf16, tag="klo")
            vlo2 = klvp.tile([P, NTL, H, HD1], bf16, tag="vlo")
            x1b = klvp.tile([P, DT, W], bf16, tag="x1b")
            nc.vector.tensor_copy(
                x1b[:, :, :].rearrange("p t w -> p (t w)"),
                x1T[:, :, :].rearrange("p t w -> p (t w)"))
            qkv_phase(x1b, qkvwT2, kv_in2, qT, klo2, vlo2, qkb2sb, vb2sb)
            nc.gpsimd.collective_compute(
                "AllGather", bass.mybir.AluOpType.bypass,
                replica_groups=groups,
                ins=[kv_in2[:]], outs=[kv_out2[:]],
            )
            if fl.pa2:
                attn_local(klo2, vlo2, None, aoT, den_a)
        with tc.tile_pool(name="kv2", bufs=1) as kvp:
            ksb2, vsb2 = load_kv(kv_out2, kvp)
            if fl.pa2:
                attn_phase(ksb2, vsb2, None, False,
                           lambda g: cb2sb[:, g : g + 1], aoT, den_b, True)
            else:
                attn_phase(ksb2, vsb2, None, fl.kb2, None, aoT, den_b, False)
        attn_norm(aoT, den_a if fl.pa2 else None, den_b, dnp)

        with tc.tile_pool(name="pr_w2", bufs=1) as wp2:
            ow2w = proj_preload(owT2, wp2)
            proj_resid_ln(ow2w, ob2sb, aoT, x1T, lns["g2"], lns["b2"], x2T)

        # ================= FFN =================
        with tc.tile_pool(name="ffh", bufs=1) as fhp, \
             tc.tile_pool(name="ffw", bufs=3) as wp, \
             tc.tile_pool(name="fft", bufs=1) as lp, \
             tc.tile_pool(name="ffpre", bufs=1) as prep:
            hT = fhp.tile([P, FT, W], bf16)
            x2b = fhp.tile([P, DT, W], bf16)
            nc.vector.tensor_copy(
                x2b[:, :, :].rearrange("p t w -> p (t w)"),
                x2T[:, :, :].rearrange("p t w -> p (t w)"))
            G4 = min(4, DT)
            FTG = min(4, FT)
            pre = prep.tile([P, DT, W], f32, tag="ffpre")
            facc = prep.tile([P, W], f32, tag="ffacc")
            fsqa = prep.tile([P, W], f32, tag="ffsqa")
            with tc.tile_pool(name="ffps1", bufs=3, space="PSUM") as psp, \
                 tc.tile_pool(name="ffps2", bufs=1, space="PSUM") as psq:
                for dg in range(DT // G4):
                    ps4 = []
                    for j in range(G4):
                        ps4j = psq.tile([P, W], f32, tag="f2ps%d" % j)
                        ps4.append(ps4j)
                    for ftg in range(FT // FTG):
                        if dg == 0:
                            w1sl = wp.tile([P, DT, FTG * P], bf16, tag="f1w")
                            nc.sync.dma_start(
                                out=w1sl[:, :, :],
                                in_=w1T[:, ftg * FTG * P : (ftg + 1) * FTG * P]
                                .rearrange("(t p) v -> p t v", p=P),
                            )
                        w2sl = wp.tile([P, FTG, G4 * P], bf16, tag="f2w")
                        nc.sync.dma_start(
                            out=w2sl[:, :, :],
                            in_=w2T[ftg * FTG * P : (ftg + 1) * FTG * P,
                                    dg * G4 * P : (dg + 1) * G4 * P]
                            .rearrange("(t p) v -> p t v", p=P),
                        )
                        for fo in range(FTG):
                            f = ftg * FTG + fo
                            if dg == 0:
                                ps = psp.tile([P, W], f32, tag="f1ps")
                                for dt in range(DT):
                                    nc.tensor.matmul(
                                        ps[:, :],
                                        lhsT=w1sl[:, dt, fo * P : (fo + 1) * P],
                                        rhs=x2b[:, dt, :],
                                        start=(dt == 0), stop=(dt == DT - 1),
                                    )
                                if fb1sb is not None:
                                    nc.scalar.activation(
                                        out=hT[:, f, :], in_=ps[:, :],
                                        func=AF.Relu,
                                        bias=fb1sb[:, f : f + 1], scale=1.0)
                                else:
                                    nc.scalar.activation(
                                        out=hT[:, f, :], in_=ps[:, :],
                                        func=AF.Relu)
                            for j in range(G4):
                                nc.tensor.matmul(
                                    ps4[j][:, :],
                                    lhsT=w2sl[:, fo, j * P : (j + 1) * P],
                                    rhs=hT[:, f, :],
                                    start=(f == 0), stop=(f == FT - 1),
                                )
                    for j in range(G4):
                        d = dg * G4 + j
                        if fb2sb is not None:
                            tmp = lp.tile([P, W], f32, tag="f2tmp")
                            nc.scalar.activation(out=tmp[:, :], in_=ps4[j][:, :],
                                                 func=AF.Identity,
                                                 bias=fb2sb[:, d : d + 1],
                                                 scale=1.0)
                            nc.vector.tensor_add(pre[:, d, :], tmp[:, :],
                                                 x2T[:, d, :])
                        else:
                            nc.vector.tensor_add(pre[:, d, :], ps4[j][:, :],
                                                 x2T[:, d, :])
                        ln_stats_step(pre, d, facc, fsqa, lp)
            with tc.tile_pool(name="fflnps", bufs=1, space="PSUM") as lpp:
                ln_t(pre, pre, lns["g3"], lns["b3"], lpp, lp, round_out=False,
                     acc=facc, sqa=fsqa)
                for d in range(DT):
                    nc.sync.dma_start(out=out[d * P : (d + 1) * P, :],
                                      in_=pre[:, d, :])


def make_program(cfg, fl):
    from concourse import bacc
    import concourse.tile as tile

    nc = bacc.Bacc("TRN2", target_bir_lowering=False, debug=False,
                   num_devices=8)
    with tile.TileContext(nc) as tc:
        _build(nc, tc, cfg, fl)
    nc.compile()
    return nc


def prep_inputs(inputs, cfg):
    """Host-side data prep. Returns (in_maps, fl)."""
    B, S, D, H, DFF, W, NTS = (cfg.B, cfg.S, cfg.D, cfg.H, cfg.DFF,
                               cfg.W, cfg.NTS)
    PR, HP = cfg.PR, cfg.HP
    import ml_dtypes
    f = np.float32
    x = np.asarray(inputs["x"], f)
    enc = np.asarray(inputs["enc_out"])
    trg = np.asarray(inputs["trg_mask"])
    fl = Flags()
    fl.qkb1 = bool(np.any(inputs["qkv_b1"]))
    fl.qkb2 = bool(np.any(inputs["qkv_b2"]))
    fl.vb1 = bool(np.any(np.asarray(inputs["qkv_b1"])[2 * D :]))
    fl.vb2 = bool(np.any(np.asarray(inputs["qkv_b2"])[2 * D :]))
    fl.ob1 = bool(np.any(inputs["out_b1"]))
    fl.ob2 = bool(np.any(inputs["out_b2"]))
    fl.fb1 = bool(np.any(inputs["ff_b1"]))
    fl.fb2 = bool(np.any(inputs["ff_b2"]))
    fl.g1 = not bool(np.all(np.asarray(inputs["ln1_g"]) == 1))
    fl.b1 = bool(np.any(inputs["ln1_b"]))
    fl.g2 = not bool(np.all(np.asarray(inputs["ln2_g"]) == 1))
    fl.b2 = bool(np.any(inputs["ln2_b"]))
    fl.g3 = not bool(np.all(np.asarray(inputs["ln3_g"]) == 1))
    fl.b3 = bool(np.any(inputs["ln3_b"]))
    fl.m1 = not bool(np.all(trg != 0))
    fl.kb2 = bool(np.any(enc == 0))

    NTL, KTG, NG = cfg.NTL, cfg.KTG, cfg.NG
    cb1_rows = {}
    if fl.m1 and NTL % KTG == 0:
        # causal-style masks: per (batch, rank), each gathered k-tile group is
        # either fully unmasked, fully masked, or the core's own quarter
        # (handled in phase A) -> exp-bias instead of multiplicative mask.
        causal_ok = True
        for b_ in range(B):
            tb_ = trg[b_] if trg.shape[0] == B else trg[0]
            for r_ in range(4):
                mt_ = (tb_[0, r_ * W : (r_ + 1) * W, :].T != 0).reshape(
                    cfg.NTS, P, W)
                row = np.zeros(NG, f)
                for g_ in range(NG):
                    lo, hi = g_ * KTG, (g_ + 1) * KTG
                    own = set(range(lo, hi)) & set(
                        range(NTL * r_, NTL * r_ + NTL))
                    if own:
                        row[g_] = f(-1e20)
                        continue
                    sub = mt_[lo:hi]
                    if sub.all():
                        row[g_] = f(0.0)
                    elif not sub.any():
                        row[g_] = f(-1e20)
                    else:
                        causal_ok = False
                cb1_rows[(b_, r_)] = row
        fl.causal = causal_ok
    else:
        fl.causal = False
    fl.pa2 = (not fl.kb2) and NTL % KTG == 0

    # selector matrix: selm[i, pr*P + p] = 1 iff i == pr*HP + p//HD
    selm = np.zeros((H, PR * P), f)
    for pr in range(PR):
        for p_ in range(P):
            selm[pr * HP + p_ // HD, pr * P + p_] = 1.0

    bfc = lambda a: np.ascontiguousarray(a.astype(ml_dtypes.bfloat16))
    shared = {
        "qkvwT1": bfc(np.asarray(inputs["qkv_w1"], f).T),
        "qkvwT2": bfc(np.asarray(inputs["qkv_w2"], f).T),
        "owT1": bfc(np.asarray(inputs["out_w1"], f).T),
        "owT2": bfc(np.asarray(inputs["out_w2"], f).T),
        "w1T": bfc(np.asarray(inputs["ff_w1"], f).T),
        "w2T": bfc(np.asarray(inputs["ff_w2"], f).T),
        "selm": selm,
    }
    if fl.qkb1:
        shared["qkvb1"] = np.asarray(inputs["qkv_b1"], f)
    if fl.qkb2:
        shared["qkvb2"] = np.asarray(inputs["qkv_b2"], f)
    if fl.vb1:
        shared["vb1"] = np.broadcast_to(
            np.asarray(inputs["qkv_b1"], f)[2 * D :], (P, D)).copy()
    if fl.vb2:
        shared["vb2"] = np.broadcast_to(
            np.asarray(inputs["qkv_b2"], f)[2 * D :], (P, D)).copy()
    if fl.ob1:
        shared["ob1"] = np.asarray(inputs["out_b1"], f)
    if fl.ob2:
        shared["ob2"] = np.asarray(inputs["out_b2"], f)
    if fl.fb1:
        shared["fb1"] = np.asarray(inputs["ff_b1"], f)
    if fl.fb2:
        shared["fb2"] = np.asarray(inputs["ff_b2"], f)
    for nm, key, use in [("g1", "ln1_g", fl.g1), ("b1", "ln1_b", fl.b1),
                         ("g2", "ln2_g", fl.g2), ("b2", "ln2_b", fl.b2),
                         ("g3", "ln3_g", fl.g3), ("b3", "ln3_b", fl.b3)]:
        if use:
            shared[nm] = np.asarray(inputs[key], f)

    xTb = [np.ascontiguousarray(x[b].T) for b in range(B)]
    in_maps = []
    for c in range(8):
        b, r = c // 4, c % 4
        m = dict(shared)
        m["xTl"] = np.ascontiguousarray(xTb[b][:, r * W : (r + 1) * W])
        if fl.m1:
            # mt[kt, i, j] = trg[0 or b, 0, r*W + j, kt*P + i]  (0/1 bf16)
            tb = trg[b] if trg.shape[0] == B else trg[0]
            blk = tb[0, r * W : (r + 1) * W, :]  # [W, S] (q, k)
            bf = ml_dtypes.bfloat16
            mt = (blk.T != 0).astype(bf).reshape(NTS, P, W)
            # phase-A mask: the core's own key quarter, [P, NTL*W]
            m["mloc"] = np.ascontiguousarray(
                np.concatenate([mt[NTL * r + lt] for lt in range(NTL)], axis=1))
            if fl.causal:
                m["cb1"] = np.ascontiguousarray(
                    np.broadcast_to(cb1_rows[(b, r)], (P, NG)).astype(f))
            else:
                # phase-B mask: own quarter zeroed (handled in phase A)
                mt = mt.copy()
                mt[NTL * r : NTL * r + NTL] = bf(0.0)
                m["m1"] = np.ascontiguousarray(
                    mt.reshape(NG, KTG, P, W).transpose(0, 2, 1, 3)
                    .reshape(NG, P, KTG * W))
        if fl.pa2:
            row2 = np.zeros(NG, f)
            for g_ in range(NG):
                if set(range(g_ * KTG, (g_ + 1) * KTG)) & set(
                        range(NTL * r, NTL * r + NTL)):
                    row2[g_] = f(-1e20)
            m["cb2"] = np.ascontiguousarray(
                np.broadcast_to(row2, (P, NG)).astype(f))
        if fl.kb2:
            eb = enc[b, 0, 0, :]  # [S]
            m["kb2"] = np.where(eb != 0, f(0.0), f(-1e20)).astype(f).reshape(
                NTS, P, 1)
        in_maps.append(m)
    return in_maps, fl


def kernel_with_results(**inputs):
    from concourse.bass_utils import run_bass_kernel_spmd

    cfg = Cfg()
    x = np.asarray(inputs["x"])
    assert x.shape == (cfg.B, cfg.S, cfg.D), x.shape
    in_maps, fl = prep_inputs(inputs, cfg)
    nc = make_program(cfg, fl)
    res = run_bass_kernel_spmd(nc, in_maps, list(range(8)))
    y = np.empty((cfg.B, cfg.S, cfg.D), np.float32)
    for c in range(8):
        b, r = c // 4, c % 4
        y[b, r * cfg.W : (r + 1) * cfg.W, :] = res.results[c]["out"].T
    return y, res


def kernel(**inputs):
    return kernel_with_results(**inputs)[0]
